# revision 24
# baseline (speedup 1.0000x reference)
"""ClipNet top-K kernel for 8 Trainium2 NeuronCores (pure data-parallel).

Math per batch row i (global i in 0..127):
  img   = normalize(input_images[i] @ W_img)            # [512]
  txt   = normalize(input_texts[i]  @ W_txt)            # [512]
  E     = other_texts[i] @ W_txt                        # [2048, 512]
  logit_oth = exp(ls) * (E @ img) / ||E||_row           # [2048]
  logit_in  = exp(ls) * (img . txt)
  out[i] = top127(logit_oth) sorted desc, with logit_in inserted at pos i

Sharding: 16 rows per core, no collectives.

Per-core pipeline:
  - other_texts shipped twice, feature-major: fp8 e4m3 (16 MB) for the
    row-norm matmul and bf16 (32 MB) for the numerator matmul. fp8 only
    perturbs ||E|| by ~0.2% (error averages over D=512), while the
    numerator needs bf16 to stay inside the 2e-2 gate.
  - E tiles [128n x 512d] via fp8 DoubleRow matmuls (2 k-tiles of 128
    per pass, 0.5 cycles/row) accumulated in PSUM.
  - ||E||^2 rowsums: Square+accum split between ScalarE (activation)
    and DVE (tensor_tensor_reduce in0=in1=E) so neither engine paces
    the loop.
  - numerators via diag-masked V matmul in bf16 (all 16 rows accumulate
    into one [16, 2048] PSUM region).
  - sorted top-128 via 16 rounds of DVE max8 + match_replace on bf16
    logits (2x DVE rate vs f32).
"""

import os
import sys

import numpy as np

sys.path.insert(0, "/opt/trn_rl_repo")

import concourse.bacc as bacc
import concourse.tile as tile
from concourse import mybir
from concourse.masks import make_identity

F32 = mybir.dt.float32
BF16 = mybir.dt.bfloat16
F8 = mybir.dt.float8e4
U8 = mybir.dt.uint8

import ml_dtypes

NP_BF16 = ml_dtypes.bfloat16
NP_F8 = ml_dtypes.float8_e4m3

B = 128
N = 2048
F_IMG = 1024
F_TXT = 512
D = 512
K = 127          # topK = B - 1
NCORES = 8
BLOC = B // NCORES   # 16 rows per core
NEG = -1e30

KC = D // 128        # 4 contraction chunks of 128
NCH = N // 128       # 16 row-chunks of 128
NG = N // 512        # 4 groups of 512 for the numerator matmul

# how many of every 16 square-reduce tiles go to the DVE-side path
# (rest on ScalarE Square+accum); tiles nch < DVE_PER16 take the DVE path
DVE_PER16 = int(os.environ.get("CLIP_SQ_DVE", "7"))
# DVE-path flavor: "bnstats" = DVE bn_stats+bn_aggr (Sum x^2 =
# (mean^2+var)*D), combined in the epilogue; "scalar" = everything on
# ScalarE Square+accum (DVE_PER16 ignored). GPSIMD/Pool cannot read
# PSUM, so the E tiles can only be consumed by ScalarE and DVE.
SQ_MODE = os.environ.get("CLIP_SQ_MODE", "bnstats")
# numerator operand precision: "bf16" ships a bf16 copy of other_texts
# (48 MB/core total); "delta8" ships fp8 + fp8 quantization residual
# (32 MB/core) and computes num = X8.(v8+dv8) + D8.v8 with DoubleRow
# matmuls (25% less PE work; rel err ~1.1e-2 vs ~6.7e-3)
NUM_MODE = os.environ.get("CLIP_NUM_MODE", "bf16")


def _build_kernel(tc):
    STAGE = int(os.environ.get("CLIP_STAGE", "4"))
    nc = tc.nc
    p = {}
    p["imgT"] = nc.declare_dram_parameter("imgT", [F_IMG, BLOC], BF16, isOutput=False)
    p["txtT"] = nc.declare_dram_parameter("txtT", [F_TXT, BLOC], BF16, isOutput=False)
    # feature-major other_texts, pre-tiled host-side as [row, p, kc, n] so
    # each SBUF partition's data is one contiguous DMA chunk (f = kc*128 + p)
    p["othT8"] = nc.declare_dram_parameter("othT8", [BLOC, 128, KC, N], F8, isOutput=False)
    if NUM_MODE == "delta8":
        p["othD8"] = nc.declare_dram_parameter("othD8", [BLOC, 128, KC, N], F8, isOutput=False)
    else:
        p["othT"] = nc.declare_dram_parameter("othT", [BLOC, 128, KC, N], BF16, isOutput=False)
    p["w_img"] = nc.declare_dram_parameter("w_img", [F_IMG, D], BF16, isOutput=False)
    p["w_txt"] = nc.declare_dram_parameter("w_txt", [F_TXT, D], BF16, isOutput=False)
    p["w_txt8"] = nc.declare_dram_parameter("w_txt8", [F_TXT, D], F8, isOutput=False)
    p["w_txtT"] = nc.declare_dram_parameter("w_txtT", [D, F_TXT], BF16, isOutput=False)
    p["m_lt"] = nc.declare_dram_parameter("m_lt", [BLOC, K + 1], U8, isOutput=False)
    p["m_eq"] = nc.declare_dram_parameter("m_eq", [BLOC, K + 1], U8, isOutput=False)
    p["ls"] = nc.declare_dram_parameter("ls", [1, 1], F32, isOutput=False)
    out_dram = nc.declare_dram_parameter("out", [BLOC, K + 1], F32, isOutput=True)

    Act = mybir.ActivationFunctionType
    Alu = mybir.AluOpType
    PM = mybir.MatmulPerfMode

    with (
        tc.tile_pool(name="weights", bufs=1) as wpool,
        tc.tile_pool(name="small", bufs=1) as small,
        tc.tile_pool(name="xt8", bufs=2) as xt8_pool,
        tc.tile_pool(name="xtb", bufs=2) as xtb_pool,
        tc.tile_pool(name="ps_e", bufs=3, space="PSUM") as ps_e,
        tc.tile_pool(name="ps_num", bufs=1, space="PSUM") as ps_num,
    ):
        # ---------------- prologue: weights + embeddings ----------------
        w_img_sb = wpool.tile([128, F_IMG // 128, D], BF16)
        nc.sync.dma_start(w_img_sb, p["w_img"][:].rearrange("(k p) d -> p k d", p=128))
        w_txt_sb = wpool.tile([128, KC, D], BF16)
        nc.sync.dma_start(w_txt_sb, p["w_txt"][:].rearrange("(k p) d -> p k d", p=128))
        w_txt8_sb = wpool.tile([128, KC, D], F8)
        nc.sync.dma_start(w_txt8_sb, p["w_txt8"][:].rearrange("(k p) d -> p k d", p=128))
        w_txtT_sb = wpool.tile([128, KC, F_TXT], BF16)
        nc.sync.dma_start(w_txtT_sb, p["w_txtT"][:].rearrange("(k p) d -> p k d", p=128))

        imgT_sb = small.tile([128, F_IMG // 128, BLOC], BF16)
        nc.sync.dma_start(imgT_sb, p["imgT"][:].rearrange("(k p) m -> p k m", p=128))
        txtT_sb = small.tile([128, KC, BLOC], BF16)
        nc.sync.dma_start(txtT_sb, p["txtT"][:].rearrange("(k p) m -> p k m", p=128))

        m_lt_sb = small.tile([BLOC, K + 1], U8)
        nc.sync.dma_start(m_lt_sb, p["m_lt"][:])
        m_eq_sb = small.tile([BLOC, K + 1], U8)
        nc.sync.dma_start(m_eq_sb, p["m_eq"][:])
        ls_sb = small.tile([1, 1], F32)
        nc.sync.dma_start(ls_sb, p["ls"][:])

        identity = small.tile([128, 128], F32)
        make_identity(nc, identity)

        # img = imgT.T @ W_img   -> [16, 512] (accumulate 8 k-chunks)
        img_ps = ps_e.tile([BLOC, D], F32, tag="tp", bufs=1)
        nkc_img = F_IMG // 128
        for k in range(nkc_img):
            nc.tensor.matmul(
                img_ps,
                lhsT=imgT_sb[:, k, :],
                rhs=w_img_sb[:, k, :],
                start=(k == 0),
                stop=(k == nkc_img - 1),
            )
        txt_ps = ps_e.tile([BLOC, D], F32, tag="tp", bufs=1)
        for k in range(KC):
            nc.tensor.matmul(
                txt_ps,
                lhsT=txtT_sb[:, k, :],
                rhs=w_txt_sb[:, k, :],
                start=(k == 0),
                stop=(k == KC - 1),
            )

        # normalize rows of img / txt (copy PSUM->SBUF first: DVE reads
        # at most one PSUM operand)
        # activations grouped by function so the ScalarE activation table is
        # swapped twice (Square set -> Ln -> Exp) instead of per-tensor
        img_sb = small.tile([BLOC, D], F32)
        nc.vector.tensor_copy(img_sb, img_ps)
        txt_sb = small.tile([BLOC, D], F32)
        nc.vector.tensor_copy(txt_sb, txt_ps)

        sq_scr = small.tile([BLOC, D], F32)
        img_nsq = small.tile([BLOC, 1], F32)
        nc.scalar.activation(sq_scr, img_sb, Act.Square, accum_out=img_nsq)
        sq_scr2 = small.tile([BLOC, D], F32)
        txt_nsq = small.tile([BLOC, 1], F32)
        nc.scalar.activation(sq_scr2, txt_sb, Act.Square, accum_out=txt_nsq)

        img_rn = small.tile([BLOC, 1], F32)
        nc.scalar.activation(img_rn, img_nsq, Act.Ln)
        txt_rn = small.tile([BLOC, 1], F32)
        nc.scalar.activation(txt_rn, txt_nsq, Act.Ln)

        # broadcast ls to [16,1] via DMA (src partition stride 0); sc16 = exp(ls)
        import concourse.bass as bass_mod
        ls_ap = p["ls"][:]
        ls_bcast = bass_mod.AP(
            tensor=ls_ap.tensor, offset=ls_ap.offset, ap=[[0, BLOC], [1, 1]]
        )
        ls16 = small.tile([BLOC, 1], F32)
        nc.sync.dma_start(ls16, ls_bcast)
        ls_bcast128 = bass_mod.AP(
            tensor=ls_ap.tensor, offset=ls_ap.offset, ap=[[0, 128], [1, 1]]
        )
        ls128 = small.tile([128, 1], F32)
        nc.sync.dma_start(ls128, ls_bcast128)

        nc.scalar.activation(img_rn, img_rn, Act.Exp, scale=-0.5)
        nc.scalar.activation(txt_rn, txt_rn, Act.Exp, scale=-0.5)
        sc16 = small.tile([BLOC, 1], F32)
        nc.scalar.activation(sc16, ls16, Act.Exp)
        sc128 = small.tile([128, 1], F32)
        nc.scalar.activation(sc128, ls128, Act.Exp)

        img_n = small.tile([BLOC, D], F32)
        nc.vector.tensor_scalar_mul(img_n, img_sb, scalar1=img_rn)
        txt_n = small.tile([BLOC, D], F32)
        nc.vector.tensor_scalar_mul(txt_n, txt_sb, scalar1=txt_rn)

        # logit_in (unscaled) = rowsum(img_n * txt_n)
        prod_it = small.tile([BLOC, D], F32)
        nc.vector.tensor_mul(prod_it, img_n, txt_n)
        sq_scr3 = small.tile([BLOC, D], F32)
        li_raw = small.tile([BLOC, 1], F32)
        nc.scalar.activation(sq_scr3, prod_it, Act.Copy, accum_out=li_raw)

        li = small.tile([BLOC, 1], F32)
        nc.vector.tensor_mul(li, li_raw, sc16)

        # img_n^T  [512, 16] via PE transposes of [16,128] slices
        imgnT_sb = small.tile([128, KC, BLOC], BF16)
        for c in range(KC):
            tp_ps = ps_e.tile([128, BLOC], F32, tag="tp", bufs=1)
            nc.tensor.transpose(tp_ps, img_n[:, 128 * c:128 * (c + 1)],
                                identity[:BLOC, :BLOC])
            nc.vector.tensor_copy(imgnT_sb[:, c, :], tp_ps)

        # V[k, b] = exp(ls) * sum_j W_txt[k, j] img_n[b, j] -> v_sb
        # (folding the logit scale into V makes logits = num / sqrt(nsq),
        # so the epilogue needs only Sqrt+divide instead of Ln/Exp)
        v_sb = small.tile([128, KC, BLOC], BF16)
        v_f32 = small.tile([128, KC, BLOC], F32)
        for kcc in range(KC):
            v_ps = ps_e.tile([128, BLOC], F32, tag="tp", bufs=1)
            for j in range(KC):
                nc.tensor.matmul(
                    v_ps,
                    lhsT=w_txtT_sb[:, j, 128 * kcc:128 * (kcc + 1)],
                    rhs=imgnT_sb[:, j, :],
                    start=(j == 0),
                    stop=(j == KC - 1),
                )
            nc.vector.tensor_copy(v_f32[:, kcc, :], v_ps)
            nc.vector.tensor_scalar_mul(v_sb[:, kcc, :], v_ps, scalar1=sc128)
        if NUM_MODE == "delta8":
            v8_sb = small.tile([128, KC, BLOC], F8)
            nc.vector.tensor_copy(v8_sb, v_f32)
            v8_back = small.tile([128, KC, BLOC], F32)
            nc.vector.tensor_copy(v8_back, v8_sb)
            dv_f32 = small.tile([128, KC, BLOC], F32)
            nc.vector.tensor_sub(dv_f32, v_f32, v8_back)
            dv8_sb = small.tile([128, KC, BLOC], F8)
            nc.vector.tensor_copy(dv8_sb, dv_f32)

        # Diagonal-masked V: v_masked[:, kc, b, j] = V[:, kc, b] iff j == b.
        # Lets all 16 rows' numerators accumulate into one [16, N] PSUM
        # region (row m of each matmul gets v_m . x only for m == b, else 0).
        zscr = small.tile([128, KC * BLOC * BLOC], F32)
        nc.vector.memset(zscr, 0.0)
        if NUM_MODE == "delta8":
            v8_m = small.tile([128, KC, BLOC, BLOC], F8)
            nc.vector.tensor_copy(v8_m.rearrange("p a b c -> p (a b c)"), zscr)
            dv8_m = small.tile([128, KC, BLOC, BLOC], F8)
            nc.vector.tensor_copy(dv8_m.rearrange("p a b c -> p (a b c)"), zscr)
            for b in range(BLOC):
                nc.sync.dma_start(v8_m[:, :, b, b:b + 1], v8_sb[:, :, b:b + 1])
                nc.sync.dma_start(dv8_m[:, :, b, b:b + 1], dv8_sb[:, :, b:b + 1])
        else:
            v_masked = small.tile([128, KC, BLOC, BLOC], BF16)
            nc.vector.tensor_copy(
                v_masked.rearrange("p a b c -> p (a b c)"), zscr
            )
            for b in range(BLOC):
                nc.sync.dma_start(v_masked[:, :, b, b:b + 1], v_sb[:, :, b:b + 1])

        if STAGE == 1:
            outt1 = small.tile([BLOC, K + 1], F32)
            nc.vector.memset(outt1, 0.0)
            nc.vector.tensor_copy(outt1[:, 0:1], li)
            nc.sync.dma_start(out_dram[:], outt1)
            return out_dram

        # ---------------- streaming loop over the 16 batch rows ----------------
        nsq_cols = small.tile([128, BLOC, NCH], F32)   # [128, 16, 16]
        if SQ_MODE == "bnstats":
            # raw BN_STATS2 output per even tile: [count, mean, M2] x 2
            # half-groups of 256; decoded in one batched epilogue pass
            stats_cols = small.tile([128, BLOC, NCH // 2, 6], F32)
        e2_pool = tc.tile_pool(name="e2", bufs=2)
        e2p = e2_pool.__enter__()
        num_ps = [
            ps_num.tile([BLOC, 512], F32, tag=f"num{g}", name=f"num_ps{g}")
            for g in range(NG)
        ]

        for b in range(BLOC):
            xt8 = xt8_pool.tile([128, KC, N], F8, tag="xt8", name=f"xt8_{b}")
            nc.sync.dma_start(xt8, p["othT8"][b])
            if NUM_MODE == "delta8":
                xd8 = xtb_pool.tile([128, KC, N], F8, tag="xd8", name=f"xd8_{b}")
                nc.sync.dma_start(xd8, p["othD8"][b])
            else:
                xtb = xtb_pool.tile([128, KC, N], BF16, tag="xtb", name=f"xtb_{b}")
                nc.sync.dma_start(xtb, p["othT"][b])

            # E tiles: [128 n, 512 d] via 2 fp8 DoubleRow matmuls (k=256 each),
            # then Square+rowsum on ScalarE or DVE -> nsq column. The
            # numerator matmul for group nch//4 is emitted after every 4th
            # E tile so the PE has independent work while the square
            # consumers drain PSUM.
            for nch in range(NCH):
                e_ps = ps_e.tile([128, D], F32, tag="e")
                for c in range(2):
                    nc.tensor.matmul(
                        e_ps,
                        lhsT=xt8[:, 2 * c:2 * c + 2, 128 * nch:128 * (nch + 1)],
                        rhs=w_txt8_sb[:, 2 * c:2 * c + 2, :],
                        start=(c == 0),
                        stop=(c == 1),
                        perf_mode=PM.DoubleRow,
                    )
                # interleave consumers (even tiles -> DVE, odd -> ScalarE) so
                # both engines drain PSUM concurrently and the PE never waits
                # behind a single slower consumer
                col = nsq_cols[:, b, nch:nch + 1]
                if nch % 2 == 0 and SQ_MODE == "bnstats":
                    nc.vector.bn_stats(stats_cols[:, b, nch // 2, :], e_ps)
                else:
                    nc.scalar.activation(e_ps, e_ps, Act.Square, accum_out=col)

                if STAGE >= 3 and nch % 4 == 3:
                    # numerator: num[b, n] = sum_k XT[k, n] V[k, b]:
                    # diag-masked lhsT zeroes rows m != b, so every b
                    # accumulates into the shared num_ps region
                    g = nch // 4
                    for kcc in range(KC):
                        nc.tensor.matmul(
                            num_ps[g],
                            lhsT=v_masked[:, kcc, b, :],
                            rhs=xtb[:, kcc, 512 * g:512 * (g + 1)],
                            start=(b == 0 and kcc == 0),
                            stop=(b == BLOC - 1 and kcc == KC - 1),
                        )

        # ---------------- epilogue ----------------
        e2_pool.__exit__(None, None, None)
        if SQ_MODE == "bnstats":
            # Sum x^2 = M2_lo + 256*mean_lo^2 + M2_hi + 256*mean_hi^2
            m_lo = stats_cols[:, :, :, 1]
            M2lo = stats_cols[:, :, :, 2]
            m_hi = stats_cols[:, :, :, 4]
            M2hi = stats_cols[:, :, :, 5]
            t0 = small.tile([128, BLOC, NCH // 2], F32)
            t1 = small.tile([128, BLOC, NCH // 2], F32)
            nc.vector.tensor_mul(t0, m_lo, m_lo)
            nc.vector.tensor_mul(t1, m_hi, m_hi)
            nc.vector.tensor_add(t0, t0, t1)
            nc.vector.tensor_scalar_mul(t0, t0, scalar1=float(D // 2))
            nc.vector.tensor_add(t0, t0, M2lo)
            nsq_even = nsq_cols.rearrange(
                "p b (n two) -> p b n two", two=2)[:, :, :, 0]
            nc.vector.tensor_add(nsq_even, t0, M2hi)

        # transpose nsq_cols [128, 256] -> per-row layout, reshape via DMA
        nsq_flat = nsq_cols.rearrange("p b n -> p (b n)")
        nsqT_sb = small.tile([128, 2, 128], F32)
        for t in range(2):
            tp2 = ps_e.tile([128, 128], F32, tag="tp", bufs=1)
            nc.tensor.transpose(tp2, nsq_flat[:, 128 * t:128 * (t + 1)], identity)
            nc.vector.tensor_copy(nsqT_sb[:, t, :], tp2)

        nsq_all = small.tile([BLOC, N], F32)
        for t in range(2):
            nc.sync.dma_start(nsq_all[8 * t:8 * (t + 1), :], nsqT_sb[:, t, :])

        # logits = (exp(ls) * num) / sqrt(nsq)  (scale already folded into V;
        # Sqrt shares the ScalarE act table with the loop's Square)
        rs = small.tile([BLOC, N], F32)
        nc.scalar.activation(rs, nsq_all, Act.Sqrt)

        if STAGE == 2:
            outt2 = small.tile([BLOC, K + 1], F32)
            nc.vector.tensor_copy(outt2, rs[:, 0:K + 1])
            nc.sync.dma_start(out_dram[:], outt2)
            return out_dram

        rq = small.tile([BLOC, N], F32)
        nc.vector.reciprocal(rq, rs)
        logits = small.tile([BLOC, N], BF16)
        for g in range(NG):
            nc.vector.tensor_mul(
                logits[:, 512 * g:512 * (g + 1)], num_ps[g],
                rq[:, 512 * g:512 * (g + 1)],
            )

        if STAGE == 3:
            outt3 = small.tile([BLOC, K + 1], F32)
            nc.vector.tensor_copy(outt3, logits[:, 0:K + 1])
            nc.sync.dma_start(out_dram[:], outt3)
            return out_dram

        # sorted top-128, two-level so DVE lanes stay busy:
        #   stage 1: spread each row over 8 partitions ([128, 256]) and take
        #   per-segment sorted top-128 (any row-top-127 element is in its
        #   segment's top-127, so this is exact);
        #   stage 2: regroup candidates to [16, 1024] and take top-128.
        l128 = small.tile([128, N // 8], BF16)
        nc.sync.dma_start(l128, logits.rearrange("b (s j) -> b s j", s=8))
        t1 = small.tile([128, 128], BF16)
        w1 = small.tile([128, N // 8], BF16)
        cur = l128
        for i in range(16):
            nc.vector.max(out=t1[:, 8 * i:8 * i + 8], in_=cur)
            nc.vector.match_replace(
                out=w1, in_to_replace=t1[:, 8 * i:8 * i + 8],
                in_values=cur, imm_value=NEG,
            )
            cur = w1

        cand = small.tile([BLOC, 8 * 128], BF16)
        t1v = t1.rearrange("(b s) j -> b s j", s=8)
        for s in range(8):
            nc.sync.dma_start(cand[:, 128 * s:128 * (s + 1)], t1v[:, s, :])

        topk_bf = small.tile([BLOC, 128], BF16)
        work = small.tile([BLOC, 8 * 128], BF16)
        cur = cand
        for i in range(16):
            nc.vector.max(out=topk_bf[:, 8 * i:8 * i + 8], in_=cur)
            nc.vector.match_replace(
                out=work,
                in_to_replace=topk_bf[:, 8 * i:8 * i + 8],
                in_values=cur,
                imm_value=NEG,
            )
            cur = work

        topk_sb = small.tile([BLOC, 128], F32)
        nc.vector.tensor_copy(topk_sb, topk_bf)

        # insert logit_in at column i (global row index): masks from host
        shifted = small.tile([BLOC, K + 1], F32)
        nc.vector.tensor_copy(shifted[:, 1:K + 1], topk_sb[:, 0:K])
        nc.vector.tensor_copy(shifted[:, 0:1], topk_sb[:, 0:1])
        outt = small.tile([BLOC, K + 1], F32)
        nc.vector.select(outt, m_lt_sb, on_true=topk_sb, on_false=shifted)
        nc.vector.copy_predicated(outt, m_eq_sb, li.to_broadcast([BLOC, K + 1]))

        nc.sync.dma_start(out_dram[:], outt)

    return out_dram


def build_module():
    nc = bacc.Bacc("TRN2", target_bir_lowering=False, debug=False, num_devices=NCORES)
    with tile.TileContext(nc) as tc:
        _build_kernel(tc)
    nc.compile()
    return nc


def make_in_maps(input_images, input_texts, other_texts, W_img, W_txt, logit_scale):
    input_images = np.asarray(input_images, np.float32)
    input_texts = np.asarray(input_texts, np.float32)
    other_texts = np.asarray(other_texts, np.float32)
    W_img = np.ascontiguousarray(np.asarray(W_img, np.float32))
    W_txt = np.ascontiguousarray(np.asarray(W_txt, np.float32))
    W_txtT = np.ascontiguousarray(W_txt.T)
    ls = np.float32(np.asarray(logit_scale).reshape(-1)[0])

    cols = np.arange(K + 1)
    in_maps = []
    for c in range(NCORES):
        r = slice(BLOC * c, BLOC * (c + 1))
        gi = np.arange(BLOC * c, BLOC * (c + 1))[:, None]  # global row ids
        # [16, 2048, 512] -> [16, f=512, n=2048] -> [16, p=128, kc=4, n]
        othT_c = np.ascontiguousarray(
            other_texts[r].transpose(0, 2, 1)
            .reshape(BLOC, KC, 128, N).transpose(0, 2, 1, 3)
        )
        oth8 = othT_c.astype(NP_F8)
        m = {
            "imgT": np.ascontiguousarray(input_images[r].T).astype(NP_BF16),
            "txtT": np.ascontiguousarray(input_texts[r].T).astype(NP_BF16),
            "othT8": oth8,
            "w_img": W_img.astype(NP_BF16),
            "w_txt": W_txt.astype(NP_BF16),
            "w_txt8": W_txt.astype(NP_F8),
            "w_txtT": W_txtT.astype(NP_BF16),
            "m_lt": (cols[None, :] < gi).astype(np.uint8),
            "m_eq": (cols[None, :] == gi).astype(np.uint8),
            "ls": np.array([[ls]], np.float32),
        }
        if NUM_MODE == "delta8":
            m["othD8"] = (othT_c - oth8.astype(np.float32)).astype(NP_F8)
        else:
            m["othT"] = othT_c.astype(NP_BF16)
        in_maps.append(m)
    return in_maps


_NC_CACHE = {}


def kernel(input_images, input_texts, other_texts, W_img, W_txt, logit_scale):
    from concourse.bass_utils import run_bass_kernel_spmd

    if "nc" not in _NC_CACHE:
        _NC_CACHE["nc"] = build_module()
    nc = _NC_CACHE["nc"]

    in_maps = make_in_maps(
        input_images, input_texts, other_texts, W_img, W_txt, logit_scale
    )
    res = run_bass_kernel_spmd(nc, in_maps, list(range(NCORES)))
    _NC_CACHE["last_result"] = res
    return np.concatenate([res.results[c]["out"] for c in range(NCORES)], axis=0)


# revision 26
# speedup vs baseline: 1.0564x; 1.0564x over previous
"""ClipNet top-K kernel for 8 Trainium2 NeuronCores (pure data-parallel).

Math per batch row i (global i in 0..127):
  img   = normalize(input_images[i] @ W_img)            # [512]
  txt   = normalize(input_texts[i]  @ W_txt)            # [512]
  E     = other_texts[i] @ W_txt                        # [2048, 512]
  logit_oth = exp(ls) * (E @ img) / ||E||_row           # [2048]
  logit_in  = exp(ls) * (img . txt)
  out[i] = top127(logit_oth) sorted desc, with logit_in inserted at pos i

Sharding: 16 rows per core, no collectives.

Per-core pipeline:
  - other_texts shipped twice, feature-major: fp8 e4m3 (16 MB) for the
    row-norm matmul and bf16 (32 MB) for the numerator matmul. fp8 only
    perturbs ||E|| by ~0.2% (error averages over D=512), while the
    numerator needs bf16 to stay inside the 2e-2 gate.
  - E tiles [128n x 512d] via fp8 DoubleRow matmuls (2 k-tiles of 128
    per pass, 0.5 cycles/row) accumulated in PSUM.
  - ||E||^2 rowsums: Square+accum split between ScalarE (activation)
    and DVE (tensor_tensor_reduce in0=in1=E) so neither engine paces
    the loop.
  - numerators via diag-masked V matmul in bf16 (all 16 rows accumulate
    into one [16, 2048] PSUM region).
  - sorted top-128 via 16 rounds of DVE max8 + match_replace on bf16
    logits (2x DVE rate vs f32).
"""

import os
import sys

import numpy as np

sys.path.insert(0, "/opt/trn_rl_repo")

import concourse.bacc as bacc
import concourse.tile as tile
from concourse import mybir
from concourse.masks import make_identity

F32 = mybir.dt.float32
BF16 = mybir.dt.bfloat16
F8 = mybir.dt.float8e4
U8 = mybir.dt.uint8

import ml_dtypes

NP_BF16 = ml_dtypes.bfloat16
NP_F8 = ml_dtypes.float8_e4m3

B = 128
N = 2048
F_IMG = 1024
F_TXT = 512
D = 512
K = 127          # topK = B - 1
NCORES = 8
BLOC = B // NCORES   # 16 rows per core
NEG = -1e30

KC = D // 128        # 4 contraction chunks of 128
NCH = N // 128       # 16 row-chunks of 128
NG = N // 512        # 4 groups of 512 for the numerator matmul

# how many of every 16 square-reduce tiles go to the DVE-side path
# (rest on ScalarE Square+accum); tiles nch < DVE_PER16 take the DVE path
DVE_PER16 = int(os.environ.get("CLIP_SQ_DVE", "7"))
# DVE-path flavor: "bnstats" = DVE bn_stats+bn_aggr (Sum x^2 =
# (mean^2+var)*D), combined in the epilogue; "scalar" = everything on
# ScalarE Square+accum (DVE_PER16 ignored). GPSIMD/Pool cannot read
# PSUM, so the E tiles can only be consumed by ScalarE and DVE.
SQ_MODE = os.environ.get("CLIP_SQ_MODE", "bnstats")
# numerator operand precision: "bf16" ships a bf16 copy of other_texts
# (48 MB/core total); "delta8" ships fp8 + fp8 quantization residual
# (32 MB/core) and computes num = X8.(v8+dv8) + D8.v8 with DoubleRow
# matmuls (25% less PE work; rel err ~1.1e-2 vs ~6.7e-3)
NUM_MODE = os.environ.get("CLIP_NUM_MODE", "bf16")


def _build_kernel(tc):
    STAGE = int(os.environ.get("CLIP_STAGE", "4"))
    nc = tc.nc
    p = {}
    p["imgT"] = nc.declare_dram_parameter("imgT", [F_IMG, BLOC], BF16, isOutput=False)
    p["txtT"] = nc.declare_dram_parameter("txtT", [F_TXT, BLOC], BF16, isOutput=False)
    # feature-major other_texts, pre-tiled host-side as [row, p, kc, n] so
    # each SBUF partition's data is one contiguous DMA chunk (f = kc*128 + p)
    p["othT8"] = nc.declare_dram_parameter("othT8", [BLOC, 128, KC, N], F8, isOutput=False)
    if NUM_MODE == "delta8":
        p["othD8"] = nc.declare_dram_parameter("othD8", [BLOC, 128, KC, N], F8, isOutput=False)
    else:
        p["othT"] = nc.declare_dram_parameter("othT", [BLOC, 128, KC, N], BF16, isOutput=False)
    p["w_img"] = nc.declare_dram_parameter("w_img", [F_IMG, D], BF16, isOutput=False)
    p["w_txt"] = nc.declare_dram_parameter("w_txt", [F_TXT, D], BF16, isOutput=False)
    p["w_txt8"] = nc.declare_dram_parameter("w_txt8", [F_TXT, D], F8, isOutput=False)
    p["w_txtT"] = nc.declare_dram_parameter("w_txtT", [D, F_TXT], BF16, isOutput=False)
    p["m_lt"] = nc.declare_dram_parameter("m_lt", [BLOC, K + 1], U8, isOutput=False)
    p["m_eq"] = nc.declare_dram_parameter("m_eq", [BLOC, K + 1], U8, isOutput=False)
    p["ls"] = nc.declare_dram_parameter("ls", [1, 1], F32, isOutput=False)
    out_dram = nc.declare_dram_parameter("out", [BLOC, K + 1], F32, isOutput=True)

    Act = mybir.ActivationFunctionType
    Alu = mybir.AluOpType
    PM = mybir.MatmulPerfMode

    with (
        tc.tile_pool(name="weights", bufs=1) as wpool,
        tc.tile_pool(name="small", bufs=1) as small,
        tc.tile_pool(name="xt8", bufs=2) as xt8_pool,
        tc.tile_pool(name="xtb", bufs=2) as xtb_pool,
        tc.tile_pool(name="ps_e", bufs=3, space="PSUM") as ps_e,
        tc.tile_pool(name="ps_num", bufs=1, space="PSUM") as ps_num,
    ):
        # ---------------- prologue: weights + embeddings ----------------
        w_img_sb = wpool.tile([128, F_IMG // 128, D], BF16)
        nc.sync.dma_start(w_img_sb, p["w_img"][:].rearrange("(k p) d -> p k d", p=128))
        w_txt_sb = wpool.tile([128, KC, D], BF16)
        nc.sync.dma_start(w_txt_sb, p["w_txt"][:].rearrange("(k p) d -> p k d", p=128))
        w_txt8_sb = wpool.tile([128, KC, D], F8)
        nc.sync.dma_start(w_txt8_sb, p["w_txt8"][:].rearrange("(k p) d -> p k d", p=128))
        w_txtT_sb = wpool.tile([128, KC, F_TXT], BF16)
        nc.sync.dma_start(w_txtT_sb, p["w_txtT"][:].rearrange("(k p) d -> p k d", p=128))

        imgT_sb = small.tile([128, F_IMG // 128, BLOC], BF16)
        nc.sync.dma_start(imgT_sb, p["imgT"][:].rearrange("(k p) m -> p k m", p=128))
        txtT_sb = small.tile([128, KC, BLOC], BF16)
        nc.sync.dma_start(txtT_sb, p["txtT"][:].rearrange("(k p) m -> p k m", p=128))

        m_lt_sb = small.tile([BLOC, K + 1], U8)
        nc.sync.dma_start(m_lt_sb, p["m_lt"][:])
        m_eq_sb = small.tile([BLOC, K + 1], U8)
        nc.sync.dma_start(m_eq_sb, p["m_eq"][:])
        ls_sb = small.tile([1, 1], F32)
        nc.sync.dma_start(ls_sb, p["ls"][:])

        identity = small.tile([128, 128], F32)
        make_identity(nc, identity)

        # img = imgT.T @ W_img   -> [16, 512] (accumulate 8 k-chunks)
        img_ps = ps_e.tile([BLOC, D], F32, tag="tp", bufs=1)
        nkc_img = F_IMG // 128
        for k in range(nkc_img):
            nc.tensor.matmul(
                img_ps,
                lhsT=imgT_sb[:, k, :],
                rhs=w_img_sb[:, k, :],
                start=(k == 0),
                stop=(k == nkc_img - 1),
            )
        txt_ps = ps_e.tile([BLOC, D], F32, tag="tp", bufs=1)
        for k in range(KC):
            nc.tensor.matmul(
                txt_ps,
                lhsT=txtT_sb[:, k, :],
                rhs=w_txt_sb[:, k, :],
                start=(k == 0),
                stop=(k == KC - 1),
            )

        # normalize rows of img / txt (copy PSUM->SBUF first: DVE reads
        # at most one PSUM operand)
        # activations grouped by function so the ScalarE activation table is
        # swapped twice (Square set -> Ln -> Exp) instead of per-tensor
        img_sb = small.tile([BLOC, D], F32)
        nc.vector.tensor_copy(img_sb, img_ps)
        txt_sb = small.tile([BLOC, D], F32)
        nc.vector.tensor_copy(txt_sb, txt_ps)

        sq_scr = small.tile([BLOC, D], F32)
        img_nsq = small.tile([BLOC, 1], F32)
        nc.scalar.activation(sq_scr, img_sb, Act.Square, accum_out=img_nsq)
        sq_scr2 = small.tile([BLOC, D], F32)
        txt_nsq = small.tile([BLOC, 1], F32)
        nc.scalar.activation(sq_scr2, txt_sb, Act.Square, accum_out=txt_nsq)

        img_rn = small.tile([BLOC, 1], F32)
        nc.scalar.activation(img_rn, img_nsq, Act.Ln)
        txt_rn = small.tile([BLOC, 1], F32)
        nc.scalar.activation(txt_rn, txt_nsq, Act.Ln)

        # broadcast ls to [16,1] via DMA (src partition stride 0); sc16 = exp(ls)
        import concourse.bass as bass_mod
        ls_ap = p["ls"][:]
        ls_bcast = bass_mod.AP(
            tensor=ls_ap.tensor, offset=ls_ap.offset, ap=[[0, BLOC], [1, 1]]
        )
        ls16 = small.tile([BLOC, 1], F32)
        nc.sync.dma_start(ls16, ls_bcast)
        ls_bcast128 = bass_mod.AP(
            tensor=ls_ap.tensor, offset=ls_ap.offset, ap=[[0, 128], [1, 1]]
        )
        ls128 = small.tile([128, 1], F32)
        nc.sync.dma_start(ls128, ls_bcast128)

        nc.scalar.activation(img_rn, img_rn, Act.Exp, scale=-0.5)
        nc.scalar.activation(txt_rn, txt_rn, Act.Exp, scale=-0.5)
        sc16 = small.tile([BLOC, 1], F32)
        nc.scalar.activation(sc16, ls16, Act.Exp)
        sc128 = small.tile([128, 1], F32)
        nc.scalar.activation(sc128, ls128, Act.Exp)

        img_n = small.tile([BLOC, D], F32)
        nc.vector.tensor_scalar_mul(img_n, img_sb, scalar1=img_rn)
        txt_n = small.tile([BLOC, D], F32)
        nc.vector.tensor_scalar_mul(txt_n, txt_sb, scalar1=txt_rn)

        # logit_in (unscaled) = rowsum(img_n * txt_n)
        prod_it = small.tile([BLOC, D], F32)
        nc.vector.tensor_mul(prod_it, img_n, txt_n)
        sq_scr3 = small.tile([BLOC, D], F32)
        li_raw = small.tile([BLOC, 1], F32)
        nc.scalar.activation(sq_scr3, prod_it, Act.Copy, accum_out=li_raw)

        li = small.tile([BLOC, 1], F32)
        nc.vector.tensor_mul(li, li_raw, sc16)

        # img_n^T  [512, 16] via PE transposes of [16,128] slices
        imgnT_sb = small.tile([128, KC, BLOC], BF16)
        for c in range(KC):
            tp_ps = ps_e.tile([128, BLOC], F32, tag="tp", bufs=1)
            nc.tensor.transpose(tp_ps, img_n[:, 128 * c:128 * (c + 1)],
                                identity[:BLOC, :BLOC])
            nc.vector.tensor_copy(imgnT_sb[:, c, :], tp_ps)

        # V[k, b] = exp(ls) * sum_j W_txt[k, j] img_n[b, j] -> v_sb
        # (folding the logit scale into V makes logits = num / sqrt(nsq),
        # so the epilogue needs only Sqrt+divide instead of Ln/Exp)
        v_sb = small.tile([128, KC, BLOC], BF16)
        v_f32 = small.tile([128, KC, BLOC], F32)
        for kcc in range(KC):
            v_ps = ps_e.tile([128, BLOC], F32, tag="tp", bufs=1)
            for j in range(KC):
                nc.tensor.matmul(
                    v_ps,
                    lhsT=w_txtT_sb[:, j, 128 * kcc:128 * (kcc + 1)],
                    rhs=imgnT_sb[:, j, :],
                    start=(j == 0),
                    stop=(j == KC - 1),
                )
            nc.vector.tensor_copy(v_f32[:, kcc, :], v_ps)
            nc.vector.tensor_scalar_mul(v_sb[:, kcc, :], v_ps, scalar1=sc128)
        if NUM_MODE == "delta8":
            v8_sb = small.tile([128, KC, BLOC], F8)
            nc.vector.tensor_copy(v8_sb, v_f32)
            v8_back = small.tile([128, KC, BLOC], F32)
            nc.vector.tensor_copy(v8_back, v8_sb)
            dv_f32 = small.tile([128, KC, BLOC], F32)
            nc.vector.tensor_sub(dv_f32, v_f32, v8_back)
            dv8_sb = small.tile([128, KC, BLOC], F8)
            nc.vector.tensor_copy(dv8_sb, dv_f32)

        # Diagonal-masked V: v_masked[:, kc, b, j] = V[:, kc, b] iff j == b.
        # Lets all 16 rows' numerators accumulate into one [16, N] PSUM
        # region (row m of each matmul gets v_m . x only for m == b, else 0).
        zscr = small.tile([128, KC * BLOC * BLOC], F32)
        nc.vector.memset(zscr, 0.0)
        if NUM_MODE == "delta8":
            v8_m = small.tile([128, KC, BLOC, BLOC], F8)
            nc.vector.tensor_copy(v8_m.rearrange("p a b c -> p (a b c)"), zscr)
            dv8_m = small.tile([128, KC, BLOC, BLOC], F8)
            nc.vector.tensor_copy(dv8_m.rearrange("p a b c -> p (a b c)"), zscr)
            for b in range(BLOC):
                nc.sync.dma_start(v8_m[:, :, b, b:b + 1], v8_sb[:, :, b:b + 1])
                nc.sync.dma_start(dv8_m[:, :, b, b:b + 1], dv8_sb[:, :, b:b + 1])
        else:
            v_masked = small.tile([128, KC, BLOC, BLOC], BF16)
            nc.vector.tensor_copy(
                v_masked.rearrange("p a b c -> p (a b c)"), zscr
            )
            for b in range(BLOC):
                nc.sync.dma_start(v_masked[:, :, b, b:b + 1], v_sb[:, :, b:b + 1])

        if STAGE == 1:
            outt1 = small.tile([BLOC, K + 1], F32)
            nc.vector.memset(outt1, 0.0)
            nc.vector.tensor_copy(outt1[:, 0:1], li)
            nc.sync.dma_start(out_dram[:], outt1)
            return out_dram

        # ---------------- streaming loop over the 16 batch rows ----------------
        nsq_cols = small.tile([128, BLOC, NCH], F32)   # [128, 16, 16]
        if SQ_MODE == "bnstats":
            # raw BN_STATS2 output per even tile: [count, mean, M2] x 2
            # half-groups of 256; decoded in one batched epilogue pass
            stats_cols = small.tile([128, BLOC, NCH // 2, 6], F32)
        e2_pool = tc.tile_pool(name="e2", bufs=2)
        e2p = e2_pool.__enter__()
        num_ps = [
            ps_num.tile([BLOC, 512], F32, tag=f"num{g}", name=f"num_ps{g}")
            for g in range(NG)
        ]

        for b in range(BLOC):
            xt8 = xt8_pool.tile([128, KC, N], F8, tag="xt8", name=f"xt8_{b}")
            nc.sync.dma_start(xt8, p["othT8"][b])
            if NUM_MODE == "delta8":
                xd8 = xtb_pool.tile([128, KC, N], F8, tag="xd8", name=f"xd8_{b}")
                nc.sync.dma_start(xd8, p["othD8"][b])
            else:
                xtb = xtb_pool.tile([128, KC, N], BF16, tag="xtb", name=f"xtb_{b}")
                nc.sync.dma_start(xtb, p["othT"][b])

            # E tiles: [128 n, 512 d] via 2 fp8 DoubleRow matmuls (k=256 each),
            # then Square+rowsum on ScalarE or DVE -> nsq column. The
            # numerator matmul for group nch//4 is emitted after every 4th
            # E tile so the PE has independent work while the square
            # consumers drain PSUM.
            for nch in range(NCH):
                e_ps = ps_e.tile([128, D], F32, tag="e")
                for c in range(2):
                    nc.tensor.matmul(
                        e_ps,
                        lhsT=xt8[:, 2 * c:2 * c + 2, 128 * nch:128 * (nch + 1)],
                        rhs=w_txt8_sb[:, 2 * c:2 * c + 2, :],
                        start=(c == 0),
                        stop=(c == 1),
                        perf_mode=PM.DoubleRow,
                    )
                # interleave consumers (even tiles -> DVE, odd -> ScalarE) so
                # both engines drain PSUM concurrently and the PE never waits
                # behind a single slower consumer
                col = nsq_cols[:, b, nch:nch + 1]
                if nch % 2 == 0 and SQ_MODE == "bnstats":
                    nc.vector.bn_stats(stats_cols[:, b, nch // 2, :], e_ps)
                else:
                    nc.scalar.activation(e_ps, e_ps, Act.Square, accum_out=col)

                if STAGE >= 3 and nch % 4 == 3:
                    # numerator: num[b, n] = sum_k XT[k, n] V[k, b]:
                    # diag-masked lhsT zeroes rows m != b, so every b
                    # accumulates into the shared num_ps region
                    g = nch // 4
                    for kcc in range(KC):
                        nc.tensor.matmul(
                            num_ps[g],
                            lhsT=v_masked[:, kcc, b, :],
                            rhs=xtb[:, kcc, 512 * g:512 * (g + 1)],
                            start=(b == 0 and kcc == 0),
                            stop=(b == BLOC - 1 and kcc == KC - 1),
                        )

        # ---------------- epilogue ----------------
        e2_pool.__exit__(None, None, None)
        if SQ_MODE == "bnstats":
            # Sum x^2 = M2_lo + 256*mean_lo^2 + M2_hi + 256*mean_hi^2
            m_lo = stats_cols[:, :, :, 1]
            M2lo = stats_cols[:, :, :, 2]
            m_hi = stats_cols[:, :, :, 4]
            M2hi = stats_cols[:, :, :, 5]
            t0 = small.tile([128, BLOC, NCH // 2], F32)
            t1 = small.tile([128, BLOC, NCH // 2], F32)
            nc.vector.tensor_mul(t0, m_lo, m_lo)
            nc.vector.tensor_mul(t1, m_hi, m_hi)
            nc.vector.tensor_add(t0, t0, t1)
            nc.vector.tensor_scalar_mul(t0, t0, scalar1=float(D // 2))
            nc.vector.tensor_add(t0, t0, M2lo)
            nsq_even = nsq_cols.rearrange(
                "p b (n two) -> p b n two", two=2)[:, :, :, 0]
            nc.vector.tensor_add(nsq_even, t0, M2hi)

        # rq = 1/sqrt(nsq), computed in the [128, 256] column layout where
        # all DVE/ScalarE lanes are busy (8x cheaper than after the
        # transpose), then transposed to per-row layout via PE + DMA
        sq_cols = small.tile([128, BLOC * NCH], F32)
        nsq_flat = nsq_cols.rearrange("p b n -> p (b n)")
        nc.scalar.activation(sq_cols, nsq_flat, Act.Sqrt)
        rq_cols = small.tile([128, BLOC * NCH], F32)
        nc.vector.reciprocal(rq_cols, sq_cols)
        nsqT_sb = small.tile([128, 2, 128], F32)
        for t in range(2):
            tp2 = ps_e.tile([128, 128], F32, tag="tp", bufs=1)
            nc.tensor.transpose(tp2, rq_cols[:, 128 * t:128 * (t + 1)], identity)
            nc.vector.tensor_copy(nsqT_sb[:, t, :], tp2)

        nsq_all = small.tile([BLOC, N], F32)
        for t in range(2):
            nc.sync.dma_start(nsq_all[8 * t:8 * (t + 1), :], nsqT_sb[:, t, :])

        # logits = (exp(ls) * num) * rq   (scale folded into V, rq = 1/sqrt)
        if STAGE == 2:
            outt2 = small.tile([BLOC, K + 1], F32)
            nc.vector.tensor_copy(outt2, nsq_all[:, 0:K + 1])
            nc.sync.dma_start(out_dram[:], outt2)
            return out_dram

        logits = small.tile([BLOC, N], BF16)
        for g in range(NG):
            nc.vector.tensor_mul(
                logits[:, 512 * g:512 * (g + 1)], num_ps[g],
                nsq_all[:, 512 * g:512 * (g + 1)],
            )

        if STAGE == 3:
            outt3 = small.tile([BLOC, K + 1], F32)
            nc.vector.tensor_copy(outt3, logits[:, 0:K + 1])
            nc.sync.dma_start(out_dram[:], outt3)
            return out_dram

        # sorted top-128, two-level so DVE lanes stay busy:
        #   stage 1: spread each row over 8 partitions ([128, 256]) and take
        #   per-segment sorted top-128 (any row-top-127 element is in its
        #   segment's top-127, so this is exact);
        #   stage 2: regroup candidates to [16, 1024] and take top-128.
        l128 = small.tile([128, N // 8], BF16)
        nc.sync.dma_start(l128, logits.rearrange("b (s j) -> b s j", s=8))
        t1 = small.tile([128, 128], BF16)
        w1 = small.tile([128, N // 8], BF16)
        cur = l128
        for i in range(16):
            nc.vector.max(out=t1[:, 8 * i:8 * i + 8], in_=cur)
            nc.vector.match_replace(
                out=w1, in_to_replace=t1[:, 8 * i:8 * i + 8],
                in_values=cur, imm_value=NEG,
            )
            cur = w1

        cand = small.tile([BLOC, 8 * 128], BF16)
        t1v = t1.rearrange("(b s) j -> b s j", s=8)
        for s in range(8):
            nc.sync.dma_start(cand[:, 128 * s:128 * (s + 1)], t1v[:, s, :])

        topk_bf = small.tile([BLOC, 128], BF16)
        work = small.tile([BLOC, 8 * 128], BF16)
        cur = cand
        for i in range(16):
            nc.vector.max(out=topk_bf[:, 8 * i:8 * i + 8], in_=cur)
            nc.vector.match_replace(
                out=work,
                in_to_replace=topk_bf[:, 8 * i:8 * i + 8],
                in_values=cur,
                imm_value=NEG,
            )
            cur = work

        topk_sb = small.tile([BLOC, 128], F32)
        nc.vector.tensor_copy(topk_sb, topk_bf)

        # insert logit_in at column i (global row index): masks from host
        shifted = small.tile([BLOC, K + 1], F32)
        nc.vector.tensor_copy(shifted[:, 1:K + 1], topk_sb[:, 0:K])
        nc.vector.tensor_copy(shifted[:, 0:1], topk_sb[:, 0:1])
        outt = small.tile([BLOC, K + 1], F32)
        nc.vector.select(outt, m_lt_sb, on_true=topk_sb, on_false=shifted)
        nc.vector.copy_predicated(outt, m_eq_sb, li.to_broadcast([BLOC, K + 1]))

        nc.sync.dma_start(out_dram[:], outt)

    return out_dram


def build_module():
    nc = bacc.Bacc("TRN2", target_bir_lowering=False, debug=False, num_devices=NCORES)
    with tile.TileContext(nc) as tc:
        _build_kernel(tc)
    nc.compile()
    return nc


def make_in_maps(input_images, input_texts, other_texts, W_img, W_txt, logit_scale):
    input_images = np.asarray(input_images, np.float32)
    input_texts = np.asarray(input_texts, np.float32)
    other_texts = np.asarray(other_texts, np.float32)
    W_img = np.ascontiguousarray(np.asarray(W_img, np.float32))
    W_txt = np.ascontiguousarray(np.asarray(W_txt, np.float32))
    W_txtT = np.ascontiguousarray(W_txt.T)
    ls = np.float32(np.asarray(logit_scale).reshape(-1)[0])

    cols = np.arange(K + 1)
    in_maps = []
    for c in range(NCORES):
        r = slice(BLOC * c, BLOC * (c + 1))
        gi = np.arange(BLOC * c, BLOC * (c + 1))[:, None]  # global row ids
        # [16, 2048, 512] -> [16, f=512, n=2048] -> [16, p=128, kc=4, n]
        othT_c = np.ascontiguousarray(
            other_texts[r].transpose(0, 2, 1)
            .reshape(BLOC, KC, 128, N).transpose(0, 2, 1, 3)
        )
        oth8 = othT_c.astype(NP_F8)
        m = {
            "imgT": np.ascontiguousarray(input_images[r].T).astype(NP_BF16),
            "txtT": np.ascontiguousarray(input_texts[r].T).astype(NP_BF16),
            "othT8": oth8,
            "w_img": W_img.astype(NP_BF16),
            "w_txt": W_txt.astype(NP_BF16),
            "w_txt8": W_txt.astype(NP_F8),
            "w_txtT": W_txtT.astype(NP_BF16),
            "m_lt": (cols[None, :] < gi).astype(np.uint8),
            "m_eq": (cols[None, :] == gi).astype(np.uint8),
            "ls": np.array([[ls]], np.float32),
        }
        if NUM_MODE == "delta8":
            m["othD8"] = (othT_c - oth8.astype(np.float32)).astype(NP_F8)
        else:
            m["othT"] = othT_c.astype(NP_BF16)
        in_maps.append(m)
    return in_maps


_NC_CACHE = {}


def kernel(input_images, input_texts, other_texts, W_img, W_txt, logit_scale):
    from concourse.bass_utils import run_bass_kernel_spmd

    if "nc" not in _NC_CACHE:
        _NC_CACHE["nc"] = build_module()
    nc = _NC_CACHE["nc"]

    in_maps = make_in_maps(
        input_images, input_texts, other_texts, W_img, W_txt, logit_scale
    )
    res = run_bass_kernel_spmd(nc, in_maps, list(range(NCORES)))
    _NC_CACHE["last_result"] = res
    return np.concatenate([res.results[c]["out"] for c in range(NCORES)], axis=0)
